# revision 9
# baseline (speedup 1.0000x reference)
"""Trainium2 Bass kernel for nn_AttentionModel (sparse_attention).

Reference computation:
    x = emb_table[tokens]                  # [B,S,D]
    scores = x @ x^T per batch             # [B,S,S]
    out = softmax(scores) @ x              # [B,S,D]
    logits = out[:, 0, :] @ cls_w.T + cls_b

Only row 0 of the attention output is used, and that row only ever meets
cls_w, so per batch element the whole model reduces to

    q = x[0]
    s_t = <x_t, q>                 (2048 dot products of length 512)
    e = exp(s);  Z = sum(e)
    logits_c = sum_t e_t * y[tok_t, c] / Z + b_c,   y = emb_table @ cls_w^T

Device strategy (data-parallel over batch, 8 cores x 4 sequences):

  * The table is uploaded as fp8(emb*32) [32000, 512] (512B rows).
    dma_gather(transpose=True) fetches each sequence's 2048 rows directly in
    d-major layout: XT[p, cu, t, eps] = fp8 x_t[256*cu + 2*p + eps]. Token 0
    doubles as the query column.
  * ONE gather per sequence (2048 indices). The SWDGE fixed cost (~1us of
    Q7 time per instruction) made the old 512-index chunking Pool-bound;
    at 4x2048 the gather DMA stream (4 MiB/core, the hard 360 GB/s floor)
    is the pacer and Pool descriptor generation hides under it.
  * Indices upload is split: sequence 0's indices go first on the SP HWDGE
    queue so gather 0's descriptor generation starts as early as possible;
    the rest rides the Activation HWDGE queue in parallel.
  * Scores run on the PE as stationary-weight matmuls (contraction dim d on
    partitions, 128-token output columns), psum-accumulated over (cu, eps).
    All four sequences' score matmuls are emitted before any softmax
    consumer so the in-order PE stream never stalls on the Activation
    engine: seq b's exp overlaps seq b+1's scores.
  * exp + per-partition softmax sums happen in one scalar-engine activation
    reading psum (scale folds away the fp8 *32 scaling).
  * y = emb @ cls_w^T is host-precomputed weight prep; the per-token y pairs
    (32KB/core) are host-laid-out token-major alongside the indices, and the
    softmax numerator sum_t e_t y_t is 16 accumulating [128,1]x[128,2]
    matmuls. Z before the numerator on the PE stream so the DVE reciprocal
    overlaps the numerator matmuls.
  * All four sequences' logits collect in one [1, 8] tile and leave in a
    single output DMA.
"""

import numpy as np

import bass_rust

import concourse.bass as bass
import concourse.mybir as mybir
import concourse.tile as tile
from concourse.bass_utils import run_bass_kernel_spmd


def _split_multiwaits(nc: bass.Bass) -> None:
    """Workaround for the walrus build in this container, which rejects
    instructions carrying more than one sync-wait command ("Too many sync
    wait commands" / "ISA wrong length" in CoreV3GenImpl setupSyncWait).

    Moves each instruction's sync waits onto dedicated single-wait NOPs
    inserted right before it on the same engine stream (bass_nofuse so
    walrus's nop-fusion can't merge them back)."""
    counter = 0
    fn = nc.m.functions[0]
    for bb in fn.blocks:
        insts = bb.instructions
        new_list = []
        changed = False
        for inst in insts:
            si = inst.sync_info
            waits = list(si.on_wait) if si is not None else []
            if waits:
                for w in waits:
                    counter += 1
                    new_list.append(
                        mybir.InstNoOp(
                            name=f"waitnop-{counter}",
                            engine=inst.engine,
                            ins=[],
                            outs=[],
                            bass_nofuse=True,
                            sync_info=bass_rust.SyncInfo(on_wait=[w], on_update=[]),
                        )
                    )
                inst.sync_info = bass_rust.SyncInfo(
                    on_wait=[], on_update=list(si.on_update)
                )
                changed = True
            new_list.append(inst)
        if changed:
            bb.instructions = new_list


def _bacc_postpasses(nc: bass.Bass) -> None:
    """GPSIMD extended instructions (InstDMAGatherAnt) need their Q7 library
    load inserted and ISA payload bytes generated — Bacc does this in
    compile(); plain Bass does not."""
    from concourse.library_config import all_libraries, standard

    mask: dict = {}
    for lib in all_libraries:
        for it in lib.instructions:
            mask[it] = mask.get(it, 0) | (1 << lib.index)
    bass_rust.insert_library_loads(nc, mask, len(all_libraries), standard.index)
    mybir.codegen_inst_isa_subclasses(nc)


B, S, D, V, C = 32, 2048, 512, 32000, 2
N_CORES = 8
BPC = B // N_CORES          # sequences per core
JT = S // 128               # 16 token tiles per sequence
NCH = 4                     # gather chunks per sequence (hardware caps the
                            # per-instruction index count; 2048 idx crashes)
CH = S // NCH               # indices per gather
JPC = CH // 128             # token tiles per chunk
EMB_SCALE = 32.0            # emb is quantized as fp8(emb*32); scores carry 32^2

F32 = mybir.dt.float32
BF16 = mybir.dt.bfloat16
FP8 = mybir.dt.float8e4
I16 = mybir.dt.int16

_CACHE: dict = {}


def _build_nc() -> bass.Bass:
    nc = bass.Bass(dynamic_dma_scratch_size=2**17, num_swdge_queues=4)
    emb_d = nc.dram_tensor("emb8", [V, D], FP8, kind="ExternalInput")
    idx_d = nc.dram_tensor("idx", [128, BPC * (S // 16)], I16, kind="ExternalInput")
    cb_d = nc.dram_tensor("cls_b", [1, C], F32, kind="ExternalInput")
    yt_d = nc.dram_tensor("yt", [128, BPC * JT * C], BF16, kind="ExternalInput")
    out_d = nc.dram_tensor("out", [1, BPC * C], F32, kind="ExternalOutput")

    mult = mybir.AluOpType.mult
    add = mybir.AluOpType.add
    EXP = mybir.ActivationFunctionType.Exp
    ACT = mybir.EngineType.Activation
    IPS = S // 16            # idx columns per sequence

    with tile.TileContext(nc) as tc:
        with (
            tc.tile_pool(name="const", bufs=1) as constp,
            tc.tile_pool(name="xp", bufs=1) as xp,
            tc.tile_pool(name="sp", bufs=BPC) as sp,
            tc.tile_pool(name="ps", bufs=1, space="PSUM") as pp,
        ):
            idx = constp.tile([128, BPC * IPS], I16)
            # seq 0's indices first (SP), so gather 0 issues ASAP; the rest
            # loads concurrently on the Activation HWDGE queue.
            nc.sync.dma_start(idx[:, 0:IPS], idx_d[:, 0:IPS])
            nc.engines[ACT].dma_start(idx[:, IPS:], idx_d[:, IPS:])
            yt = constp.tile([128, BPC, JT, C], BF16)
            nc.sync.dma_start(yt[:, :, :, :], yt_d[:, :])
            cb = constp.tile([1, C], F32)
            nc.sync.dma_start(cb[:], cb_d[:, :])
            ones128 = constp.tile([128, 1], F32)
            nc.vector.memset(ones128[:], 1.0)

            # --- transpose-gather, NCH instructions per sequence ---
            # xt[p, b, g, cu, t, eps] = fp8 x_{CH*g+t}(seq b)[256*cu + 2*p + eps]
            xt = xp.tile([128, BPC, NCH, 2, CH, 2], FP8, tag="xt")
            for b in range(BPC):
                for g in range(NCH):
                    gout = (
                        xt[:, b, g, :, :, :]
                        .rearrange("p cu t e -> p (cu t e)")
                        .rearrange("p (a b) -> p a b", a=4)
                    )
                    nc.gpsimd.dma_gather(
                        out_ap=gout,
                        in_ap=emb_d[:, :],
                        idxs_ap=idx[:, b * IPS + g * (CH // 16):
                                    b * IPS + (g + 1) * (CH // 16)],
                        num_idxs=CH,
                        num_idxs_reg=CH,
                        elem_size=D,
                        transpose=True,
                        queue_num=(b * NCH + g) % 4,
                    )

            # --- scores: s[t] = <x_t, q>, q = token-0 column ---
            # All sequences' score matmuls first: the PE stream never waits
            # on a softmax consumer, so seq b's exp overlaps seq b+1 scores.
            # Per-sequence psum lives in disjoint column slices of shared
            # tiles (PSUM has only 8 banks; per-tag-per-buf banks overflow).
            spm_all = pp.tile([128, BPC, JT], F32, tag="spm")
            zpm_all = pp.tile([1, BPC], F32, tag="zpm")
            npm_all = pp.tile([1, BPC, C], F32, tag="npm")
            for b in range(BPC):
                spm = spm_all[:, b, :]
                for j in range(JT):
                    g, jj = divmod(j, JPC)
                    first = True
                    for cu in range(2):
                        for eps in range(2):
                            nc.tensor.matmul(
                                spm[:, j:j + 1],
                                xt[:, b, g, cu, 128 * jj:128 * (jj + 1), eps],
                                xt[:, b, 0, cu, 0:1, eps],
                                start=first,
                                stop=(cu == 1 and eps == 1),
                            )
                            first = False

            # --- softmax pieces + numerator per sequence ---
            ob = sp.tile([1, BPC * C], F32, tag="ob")
            for b in range(BPC):
                e = sp.tile([128, JT], BF16, tag="e")
                zcol = sp.tile([128, 1], F32, tag="zcol")
                nc.scalar.activation(
                    e[:], spm_all[:, b, :], EXP,
                    scale=1.0 / (EMB_SCALE * EMB_SCALE),
                    accum_out=zcol[:],
                )
                # Z first so the DVE reciprocal overlaps the numerator chain.
                zpm = zpm_all[:, b:b + 1]
                nc.tensor.matmul(zpm, zcol[:], ones128[:], start=True, stop=True)
                npm = npm_all[:, b, :]
                for j in range(JT):
                    nc.tensor.matmul(
                        npm, e[:, j:j + 1], yt[:, b, j, :],
                        start=(j == 0), stop=(j == JT - 1),
                    )
                rz = sp.tile([1, 1], F32, tag="rz")
                nc.vector.reciprocal(rz[:], zpm)
                nc.vector.scalar_tensor_tensor(
                    ob[:, b * C:(b + 1) * C], npm, rz[:], cb[:], op0=mult, op1=add
                )

            nc.sync.dma_start(out_d[:, :], ob[:])

    nc.finalize()
    _bacc_postpasses(nc)
    _split_multiwaits(nc)
    return nc


def get_nc() -> bass.Bass:
    if "nc" not in _CACHE:
        _CACHE["nc"] = _build_nc()
    return _CACHE["nc"]


def _prep_tables(emb_table: np.ndarray, cls_w: np.ndarray):
    import ml_dtypes

    emb = np.asarray(emb_table, dtype=np.float32)
    y = (emb @ np.asarray(cls_w, dtype=np.float32).T).astype(ml_dtypes.bfloat16)
    emb8 = (emb * EMB_SCALE).astype(ml_dtypes.float8_e4m3fn)
    return emb8, y


def _build_yt(tokens: np.ndarray, y: np.ndarray) -> np.ndarray:
    """Per-core [128, BPC*JT*C] bf16: yt[p, b, j, c] = y[tok[b, 128*j + p], c]
    (token-major, matching the gather's score-column layout)."""
    out = np.empty((128, BPC, JT, C), y.dtype)
    for b in range(BPC):
        out[:, b] = y[tokens[b]].reshape(JT, 128, C).transpose(1, 0, 2)
    return out.reshape(128, BPC * JT * C)


def _build_idx(tokens: np.ndarray) -> np.ndarray:
    """Per-core [128, BPC*128] int16; token t of sequence b sits at
    [16*g + t%16, b*128 + t//16] for every 16-partition group g (the SWDGE
    TX core reads group 1; CoreSim reads group 0)."""
    toks = tokens.astype(np.int16)          # [BPC, S], values < 32000
    slot = np.empty((16, BPC * (S // 16)), np.int16)
    for b in range(BPC):
        slot[:, b * (S // 16):(b + 1) * (S // 16)] = toks[b].reshape(S // 16, 16).T
    return np.tile(slot, (8, 1))


def make_in_maps(tokens, emb_table, cls_w, cls_b):
    tokens = np.asarray(tokens)
    emb8, y = _prep_tables(emb_table, cls_w)
    cb = np.ascontiguousarray(np.asarray(cls_b, dtype=np.float32)).reshape(1, C)
    in_maps = []
    for core in range(N_CORES):
        ct = tokens[core * BPC:(core + 1) * BPC]
        in_maps.append(
            {
                "emb8": emb8,
                "idx": _build_idx(ct),
                "yt": _build_yt(ct, y),
                "cls_b": cb,
            }
        )
    return in_maps


def kernel(tokens, emb_table, cls_w, cls_b) -> np.ndarray:
    nc = get_nc()
    in_maps = make_in_maps(tokens, emb_table, cls_w, cls_b)
    res = run_bass_kernel_spmd(nc, in_maps, core_ids=list(range(N_CORES)))
    outs = [res.results[c]["out"].reshape(BPC, C) for c in range(N_CORES)]
    return np.concatenate(outs, axis=0).astype(np.float32)


# revision 11
# speedup vs baseline: 1.2981x; 1.2981x over previous
"""Trainium2 Bass kernel for nn_AttentionModel (sparse_attention).

Reference computation:
    x = emb_table[tokens]                  # [B,S,D]
    scores = x @ x^T per batch             # [B,S,S]
    out = softmax(scores) @ x              # [B,S,D]
    logits = out[:, 0, :] @ cls_w.T + cls_b

Only row 0 of the attention output is used, and that row only ever meets
cls_w, so per batch element the whole model reduces to

    q = x[0]
    s_t = <x_t, q>                 (2048 dot products of length 512)
    e = exp(s);  Z = sum(e)
    logits_c = sum_t e_t * y[tok_t, c] / Z + b_c,   y = emb_table @ cls_w^T

Device strategy (data-parallel over batch, 8 cores x 4 sequences = 8192
tokens per core, treated as one global token stream):

  * The table is uploaded as fp8(emb*32) [32000, 512] (512B rows).
    dma_gather(transpose=True) fetches rows in d-major layout; within a
    chunk of n tokens the SBUF bytes are [p, v(2), i(n), e(2)] =
    fp8 x_i[256*v + 2*p + e]. Token 2048*b doubles as sequence b's query.
  * Gathers use single_packet=False on ONE SWDGE queue: that combination
    is correct on hardware for arbitrarily large num_idxs (verified to
    8192), unlike single_packet=True (crashes above 512 indices) or
    multi-queue single_packet=False (queues corrupt each other). Large
    chunks amortize the ~1us SWDGE fixed cost per instruction, so the
    gather DMA stream (4 MiB/core at the hard 360 GB/s ceiling) is the
    pacer, not Q7 descriptor generation.
  * Chunk sizes ramp [1280, 2304, 2304, 1792, 512]: the first chunk is
    small enough to start the DMA stream early, later chunks are big
    enough that descriptor generation stays ahead of the DMA engines, and
    the tiny last chunk keeps the end-of-stream softmax work short.
  * Index upload is split: the first chunk's indices ride the SP HWDGE
    queue, the rest ride the Activation HWDGE queue concurrently.
  * Scores run on the PE as stationary-weight matmuls (contraction dim d
    on partitions, 128-token output columns), psum-accumulated over
    (v, e). Emission is chunk-major with each sequence's softmax
    consumers placed between score bursts, so the in-order PE stream
    never stalls: seq b's numerator overlaps seq b+1's scores.
  * exp + per-partition softmax sums happen in one scalar-engine
    activation reading psum (scale folds away the fp8 *32 scaling). The
    last sequence's exp/numerator is split at the final chunk boundary so
    only 512 tokens' worth of softmax work trails the last gather.
  * y = emb @ cls_w^T is host-precomputed weight prep; the per-token y
    pairs (32KB/core) are host-laid-out token-major alongside the
    indices, and the softmax numerator sum_t e_t y_t is 16 accumulating
    [128,1]x[128,2] matmuls per sequence. Z's matmul precedes the
    numerator chain so the DVE reciprocal overlaps it.
  * All four sequences' logits collect in one [1, 8] tile and leave in a
    single output DMA.
"""

import numpy as np

import bass_rust

import concourse.bass as bass
import concourse.mybir as mybir
import concourse.tile as tile
from concourse.bass_utils import run_bass_kernel_spmd


def _split_multiwaits(nc: bass.Bass) -> None:
    """Workaround for the walrus build in this container, which rejects
    instructions carrying more than one sync-wait command ("Too many sync
    wait commands" / "ISA wrong length" in CoreV3GenImpl setupSyncWait).

    Moves each instruction's sync waits onto dedicated single-wait NOPs
    inserted right before it on the same engine stream (bass_nofuse so
    walrus's nop-fusion can't merge them back)."""
    counter = 0
    fn = nc.m.functions[0]
    for bb in fn.blocks:
        insts = bb.instructions
        new_list = []
        changed = False
        for inst in insts:
            si = inst.sync_info
            waits = list(si.on_wait) if si is not None else []
            if waits:
                for w in waits:
                    counter += 1
                    new_list.append(
                        mybir.InstNoOp(
                            name=f"waitnop-{counter}",
                            engine=inst.engine,
                            ins=[],
                            outs=[],
                            bass_nofuse=True,
                            sync_info=bass_rust.SyncInfo(on_wait=[w], on_update=[]),
                        )
                    )
                inst.sync_info = bass_rust.SyncInfo(
                    on_wait=[], on_update=list(si.on_update)
                )
                changed = True
            new_list.append(inst)
        if changed:
            bb.instructions = new_list


def _bacc_postpasses(nc: bass.Bass) -> None:
    """GPSIMD extended instructions (InstDMAGatherAnt) need their Q7 library
    load inserted and ISA payload bytes generated — Bacc does this in
    compile(); plain Bass does not."""
    from concourse.library_config import all_libraries, standard

    mask: dict = {}
    for lib in all_libraries:
        for it in lib.instructions:
            mask[it] = mask.get(it, 0) | (1 << lib.index)
    bass_rust.insert_library_loads(nc, mask, len(all_libraries), standard.index)
    mybir.codegen_inst_isa_subclasses(nc)


B, S, D, V, C = 32, 2048, 512, 32000, 2
N_CORES = 8
BPC = B // N_CORES          # sequences per core
T = BPC * S                 # tokens per core (global stream)
JT = S // 128               # 16 token tiles per sequence
EMB_SCALE = 32.0            # emb is quantized as fp8(emb*32); scores carry 32^2

# Gather chunk sizes over the global 8192-token stream (each a multiple of
# 128, summing to T). See module docstring for the rationale.
CHUNKS = [1280, 2304, 2304, 1792, 512]
assert sum(CHUNKS) == T and all(c % 128 == 0 for c in CHUNKS)
_offs = [sum(CHUNKS[:i]) for i in range(len(CHUNKS))]

F32 = mybir.dt.float32
BF16 = mybir.dt.bfloat16
FP8 = mybir.dt.float8e4
I16 = mybir.dt.int16

_CACHE: dict = {}


def _chunk_of(tok: int) -> int:
    """Chunk index containing global token `tok`."""
    for i, o in enumerate(_offs):
        if o <= tok < o + CHUNKS[i]:
            return i
    raise ValueError(tok)


def _build_nc() -> bass.Bass:
    nc = bass.Bass(dynamic_dma_scratch_size=2**17, num_swdge_queues=1)
    emb_d = nc.dram_tensor("emb8", [V, D], FP8, kind="ExternalInput")
    idx_d = nc.dram_tensor("idx", [128, T // 16], I16, kind="ExternalInput")
    cb_d = nc.dram_tensor("cls_b", [1, C], F32, kind="ExternalInput")
    yt_d = nc.dram_tensor("yt", [128, BPC * JT * C], BF16, kind="ExternalInput")
    out_d = nc.dram_tensor("out", [1, BPC * C], F32, kind="ExternalOutput")

    mult = mybir.AluOpType.mult
    add = mybir.AluOpType.add
    EXP = mybir.ActivationFunctionType.Exp
    ACT = mybir.EngineType.Activation
    ESC = 1.0 / (EMB_SCALE * EMB_SCALE)
    IC0 = CHUNKS[0] // 16    # idx columns of chunk 0

    with tile.TileContext(nc) as tc:
        with (
            tc.tile_pool(name="const", bufs=1) as constp,
            tc.tile_pool(name="xp", bufs=1) as xp,
            tc.tile_pool(name="sp", bufs=BPC + 1) as sp,
            tc.tile_pool(name="ps", bufs=1, space="PSUM") as pp,
        ):
            idx = constp.tile([128, T // 16], I16)
            # chunk 0's indices first (SP) so gather 0 issues ASAP; the rest
            # loads concurrently on the Activation HWDGE queue.
            nc.sync.dma_start(idx[:, 0:IC0], idx_d[:, 0:IC0])
            nc.engines[ACT].dma_start(idx[:, IC0:], idx_d[:, IC0:])
            yt = constp.tile([128, BPC, JT, C], BF16)
            nc.sync.dma_start(yt[:, :, :, :], yt_d[:, :])
            cb = constp.tile([1, C], F32)
            nc.sync.dma_start(cb[:], cb_d[:, :])
            ones128 = constp.tile([128, 1], F32)
            nc.vector.memset(ones128[:], 1.0)

            # --- transpose-gather the global token stream, chunk by chunk ---
            # Flat xt: chunk k (offset o, n tokens) occupies bytes
            # [4o, 4(o+n)) per partition, internally [v(2), i(n), e(2)]:
            # xt_k[p, v, i, e] = fp8 x_{o+i}[256 v + 2 p + e].
            xt = xp.tile([128, 4 * T], FP8, tag="xt")
            xv = []
            for k, (o, n) in enumerate(zip(_offs, CHUNKS)):
                xv.append(
                    xt[:, 4 * o:4 * (o + n)].rearrange(
                        "p (v i e) -> p v i e", v=2, e=2
                    )
                )
                gout = xt[:, 4 * o:4 * (o + n)].rearrange("p (a b) -> p a b", a=4)
                nc.gpsimd.dma_gather(
                    out_ap=gout,
                    in_ap=emb_d[:, :],
                    idxs_ap=idx[:, o // 16:(o + n) // 16],
                    num_idxs=n,
                    num_idxs_reg=n,
                    elem_size=D,
                    transpose=True,
                    queue_num=0,
                    single_packet=False,
                )

            def scores(b, j):
                """One 128-token tile's 4 accumulating dot-product matmuls:
                spm[:, b, j] (token 2048 b + 128 j + p) vs seq b's query."""
                t0 = S * b + 128 * j
                k = _chunk_of(t0)
                i0 = t0 - _offs[k]
                kq = _chunk_of(S * b)
                iq = S * b - _offs[kq]
                first = True
                for v in range(2):
                    for e in range(2):
                        nc.tensor.matmul(
                            spm_all[:, b, j:j + 1],
                            xv[k][:, v, i0:i0 + 128, e],
                            xv[kq][:, v, iq:iq + 1, e],
                            start=first,
                            stop=(v == 1 and e == 1),
                        )
                        first = False

            spm_all = pp.tile([128, BPC, JT], F32, tag="spm")
            zpm_all = pp.tile([1, BPC], F32, tag="zpm")
            npm_all = pp.tile([1, BPC, C], F32, tag="npm")
            ob = sp.tile([1, BPC * C], F32, tag="ob")

            def finalize_b(b):
                rz = sp.tile([1, 1], F32, tag="rz")
                nc.vector.reciprocal(rz[:], zpm_all[:, b:b + 1])
                nc.vector.scalar_tensor_tensor(
                    ob[:, b * C:(b + 1) * C], npm_all[:, b, :], rz[:], cb[:],
                    op0=mult, op1=add,
                )

            def zn_b(b, e, zcol, j0, j1, start, stop):
                """Z matmul first (the reciprocal overlaps the numerator),
                then accumulating numerator matmuls for tiles [j0, j1)."""
                nc.tensor.matmul(
                    zpm_all[:, b:b + 1], zcol, ones128[:],
                    start=start, stop=stop,
                )
                for j in range(j0, j1):
                    nc.tensor.matmul(
                        npm_all[:, b, :], e[:, j - j0:j - j0 + 1], yt[:, b, j, :],
                        start=(start and j == j0), stop=(stop and j == j1 - 1),
                    )

            # PE emission order: chunk-major score bursts with each
            # sequence's softmax consumers slotted right after the burst
            # that completes it (by then its exp has long run on ACT).
            # Seq 3 splits at the final chunk boundary (token 7680 = tile
            # JSPLIT) so only 512 tokens' softmax work trails the stream.
            JSPLIT = (_offs[-1] - 3 * S) // 128
            e3 = sp.tile([128, JT], BF16, tag="e3")
            zc3 = sp.tile([128, 2], F32, tag="zc3")
            by_chunk: list = [[] for _ in CHUNKS]
            for b in range(BPC):
                for j in range(JT):
                    by_chunk[_chunk_of(S * b + 128 * j)].append((b, j))

            emitted = set()
            for k in range(len(CHUNKS)):
                for b, j in by_chunk[k]:
                    scores(b, j)
                end = _offs[k] + CHUNKS[k]
                for b in range(BPC - 1):
                    if b not in emitted and (b + 1) * S <= end:
                        emitted.add(b)
                        e = sp.tile([128, JT], BF16, tag="e")
                        zcol = sp.tile([128, 1], F32, tag="zcol")
                        nc.scalar.activation(
                            e[:], spm_all[:, b, :], EXP,
                            scale=ESC, accum_out=zcol[:],
                        )
                        zn_b(b, e[:], zcol[:], 0, JT, True, True)
                        finalize_b(b)
                if k == len(CHUNKS) - 2:
                    # seq 3 head: exp + partial numerator for tiles < JSPLIT
                    # run while the final chunk is still landing.
                    nc.scalar.activation(
                        e3[:, 0:JSPLIT], spm_all[:, 3, 0:JSPLIT], EXP,
                        scale=ESC, accum_out=zc3[:, 0:1],
                    )
                    zn_b(3, e3[:, 0:JSPLIT], zc3[:, 0:1], 0, JSPLIT, True, False)

            # seq 3 tail: the final 512-token chunk only.
            nc.scalar.activation(
                e3[:, JSPLIT:], spm_all[:, 3, JSPLIT:], EXP,
                scale=ESC, accum_out=zc3[:, 1:2],
            )
            zn_b(3, e3[:, JSPLIT:], zc3[:, 1:2], JSPLIT, JT, False, True)
            finalize_b(3)

            nc.sync.dma_start(out_d[:, :], ob[:])

    nc.finalize()
    _bacc_postpasses(nc)
    _split_multiwaits(nc)
    return nc


def get_nc() -> bass.Bass:
    if "nc" not in _CACHE:
        _CACHE["nc"] = _build_nc()
    return _CACHE["nc"]


def _prep_tables(emb_table: np.ndarray, cls_w: np.ndarray):
    import ml_dtypes

    emb = np.asarray(emb_table, dtype=np.float32)
    y = (emb @ np.asarray(cls_w, dtype=np.float32).T).astype(ml_dtypes.bfloat16)
    emb8 = (emb * EMB_SCALE).astype(ml_dtypes.float8_e4m3fn)
    return emb8, y


def _build_yt(tokens: np.ndarray, y: np.ndarray) -> np.ndarray:
    """Per-core [128, BPC*JT*C] bf16: yt[p, b, j, c] = y[tok[b, 128*j + p], c]
    (token-major, matching the gather's score-column layout)."""
    out = np.empty((128, BPC, JT, C), y.dtype)
    for b in range(BPC):
        out[:, b] = y[tokens[b]].reshape(JT, 128, C).transpose(1, 0, 2)
    return out.reshape(128, BPC * JT * C)


def _build_idx(tokens: np.ndarray) -> np.ndarray:
    """Per-core [128, T//16] int16; global token t sits at
    [16*g + t%16, t//16] for every 16-partition group g (the SWDGE
    TX core reads group 1; CoreSim reads group 0)."""
    toks = tokens.astype(np.int16).reshape(-1)   # [T], values < 32000
    slot = toks.reshape(T // 16, 16).T           # [16, T//16]
    return np.tile(slot, (8, 1))


def make_in_maps(tokens, emb_table, cls_w, cls_b):
    tokens = np.asarray(tokens)
    emb8, y = _prep_tables(emb_table, cls_w)
    cb = np.ascontiguousarray(np.asarray(cls_b, dtype=np.float32)).reshape(1, C)
    in_maps = []
    for core in range(N_CORES):
        ct = tokens[core * BPC:(core + 1) * BPC]
        in_maps.append(
            {
                "emb8": emb8,
                "idx": _build_idx(ct),
                "yt": _build_yt(ct, y),
                "cls_b": cb,
            }
        )
    return in_maps


def kernel(tokens, emb_table, cls_w, cls_b) -> np.ndarray:
    nc = get_nc()
    in_maps = make_in_maps(tokens, emb_table, cls_w, cls_b)
    res = run_bass_kernel_spmd(nc, in_maps, core_ids=list(range(N_CORES)))
    outs = [res.results[c]["out"].reshape(BPC, C) for c in range(N_CORES)]
    return np.concatenate(outs, axis=0).astype(np.float32)


# revision 46
# speedup vs baseline: 1.4072x; 1.0840x over previous
"""Trainium2 Bass kernel for nn_AttentionModel (sparse_attention).

Reference computation:
    x = emb_table[tokens]                  # [B,S,D]
    scores = x @ x^T per batch             # [B,S,S]
    out = softmax(scores) @ x              # [B,S,D]
    logits = out[:, 0, :] @ cls_w.T + cls_b

Only row 0 of the attention output is used, and that row only ever meets
cls_w, so per batch element the whole model reduces to

    q = x[0]
    s_t = <x_t, q>                 (2048 dot products of length 512)
    e = exp(s);  Z = sum(e)
    logits_c = sum_t e_t * y[tok_t, c] / Z + b_c,   y = emb_table @ cls_w^T

Device strategy (data-parallel over batch, 8 cores x 4 sequences = 8192
tokens per core, treated as one global token stream):

  * The table is uploaded as fp8(emb*32) [32000, 512] (512B rows).
    dma_gather(transpose=True) fetches rows in d-major layout; within a
    chunk of n tokens the SBUF bytes are [p, v(2), i(n), e(2)] =
    fp8 x_i[256*v + 2*p + e]. Token 2048*b doubles as sequence b's query.
  * Gathers use single_packet=False on ONE SWDGE queue: that combination
    is correct on hardware for arbitrarily large num_idxs (verified to
    8192), unlike single_packet=True (crashes above 512 indices) or
    multi-queue single_packet=False (queues corrupt each other). Large
    chunks amortize the ~1us SWDGE fixed cost per instruction, so the
    gather DMA stream (4 MiB/core at the hard 360 GB/s ceiling) is the
    pacer, not Q7 descriptor generation.
  * Chunk sizes ramp [1280, 2304, 2560, 1408, 640]: the first chunk is
    the smallest that keeps descriptor generation ahead of the DMA
    engines (994 + 0.34 n_next <= 1.42 n ns), chunk boundaries align
    with sequence ends so each sequence's softmax is gated as early as
    possible, and the small last chunk keeps the end-of-stream softmax
    work to five tiles of one sequence.
  * The index upload (split SP / Activation HWDGE queues) is hoisted
    ahead of the TileContext entry barrier by a post-pass, so its
    ~2.4us latency chain overlaps the framework's entry sync instead of
    following it.
  * Scores run on the PE as stationary-weight matmuls (contraction dim d
    on partitions, 128-token output columns), psum-accumulated over
    (v, e). Emission is chunk-major with softmax consumers slotted
    between score bursts at points where their exp results are already
    available, so the in-order PE stream never stalls.
  * exp happens on the scalar engine reading psum (scale folds away the
    fp8 *32 scaling); sequences gated by the same chunk share one
    activation instruction. Z rides the numerator matmuls as a third
    ones-column in the host-prepped y table, so sum_t e_t * (y0, y1, 1)
    is one accumulating [128,1]x[128,3] matmul chain per sequence and
    logits finish with a DVE reciprocal + multiply-add. The last
    sequence's softmax splits at the final chunk boundary so little work
    trails the last gather, and numerator chains lag their score bursts
    by two chunks so the in-order PE stream never stalls a burst.
  * All four sequences' logits collect in one [1, 8] tile and leave in a
    single output DMA.
"""

import numpy as np

import bass_rust

import concourse.bass as bass
import concourse.mybir as mybir
import concourse.tile as tile
from concourse.bass_utils import run_bass_kernel_spmd


def _split_multiwaits(nc: bass.Bass) -> None:
    """Workaround for the walrus build in this container, which rejects
    instructions carrying more than one sync-wait command ("Too many sync
    wait commands" / "ISA wrong length" in CoreV3GenImpl setupSyncWait).

    Moves each instruction's sync waits onto dedicated single-wait NOPs
    inserted right before it on the same engine stream (bass_nofuse so
    walrus's nop-fusion can't merge them back)."""
    counter = 0
    fn = nc.m.functions[0]
    for bb in fn.blocks:
        insts = bb.instructions
        new_list = []
        changed = False
        for inst in insts:
            si = inst.sync_info
            waits = list(si.on_wait) if si is not None else []
            if waits:
                for w in waits:
                    counter += 1
                    new_list.append(
                        mybir.InstNoOp(
                            name=f"waitnop-{counter}",
                            engine=inst.engine,
                            ins=[],
                            outs=[],
                            bass_nofuse=True,
                            sync_info=bass_rust.SyncInfo(on_wait=[w], on_update=[]),
                        )
                    )
                inst.sync_info = bass_rust.SyncInfo(
                    on_wait=[], on_update=list(si.on_update)
                )
                changed = True
            new_list.append(inst)
        if changed:
            bb.instructions = new_list


def _hoist_idx_dmas(nc: bass.Bass) -> None:
    """Move the two index-upload DMACopies (they read the DRAM tensor
    named "idx") ahead of the TileContext entry barrier: out of their
    scheduled block and into the preamble block, right after their
    engine's RegisterMove run. Their framework waits are stripped (they
    are the first writes to the idx tile); their completion-sem
    increments stay, which is all the gathers wait on."""
    fn = nc.m.functions[0]
    blocks = fn.blocks
    moved = []
    for bb in blocks:
        keep = []
        for inst in bb.instructions:
            is_idx_dma = isinstance(inst, mybir.InstDMACopy) and any(
                getattr(ap, "memref", None) == "idx" for ap in list(inst.ins)
            )
            if is_idx_dma:
                si = inst.sync_info
                ups = list(si.on_update) if si is not None else []
                inst.sync_info = bass_rust.SyncInfo(on_wait=[], on_update=ups)
                moved.append(inst)
            else:
                keep.append(inst)
        if len(keep) != len(bb.instructions):
            bb.instructions = keep
    assert len(moved) == 2, f"expected 2 idx DMAs, found {len(moved)}"
    b0 = blocks[0]
    insts = list(b0.instructions)
    for inst in moved:
        last_rm = max(
            i for i, x in enumerate(insts)
            if isinstance(x, mybir.InstRegisterMove) and x.engine == inst.engine
        )
        insts.insert(last_rm + 1, inst)
    b0.instructions = insts


def _bacc_postpasses(nc: bass.Bass) -> None:
    """GPSIMD extended instructions (InstDMAGatherAnt) need their Q7 library
    load inserted and ISA payload bytes generated — Bacc does this in
    compile(); plain Bass does not."""
    from concourse.library_config import all_libraries, standard

    mask: dict = {}
    for lib in all_libraries:
        for it in lib.instructions:
            mask[it] = mask.get(it, 0) | (1 << lib.index)
    bass_rust.insert_library_loads(nc, mask, len(all_libraries), standard.index)
    mybir.codegen_inst_isa_subclasses(nc)


B, S, D, V, C = 32, 2048, 512, 32000, 2
CY = C + 1                  # y columns + the folded-Z ones column
N_CORES = 8
BPC = B // N_CORES          # sequences per core
T = BPC * S                 # tokens per core (global stream)
JT = S // 128               # 16 token tiles per sequence
EMB_SCALE = 32.0            # emb is quantized as fp8(emb*32); scores carry 32^2

# STREAM lists 128-token tiles (b, j) in gather order (natural order
# measured fastest end-to-end); CHUNKS slices the stream into gather
# instructions (each a multiple of 128, summing to T; sized so SWDGE
# descriptor generation stays ahead of the DMA engines:
# 994 + 0.34 n_next <= 1.42 n ns, with boundaries on sequence ends).
STREAM = [(b, j) for b in range(BPC) for j in range(JT)]
_POS = {bj: 128 * i for i, bj in enumerate(STREAM)}   # (b, j) -> stream token
CHUNKS = [1280, 2304, 2560, 1408, 640]
assert sum(CHUNKS) == T and all(c % 128 == 0 for c in CHUNKS)
_offs = [sum(CHUNKS[:i]) for i in range(len(CHUNKS))]

F32 = mybir.dt.float32
BF16 = mybir.dt.bfloat16
FP8 = mybir.dt.float8e4
I16 = mybir.dt.int16

_CACHE: dict = {}


def _chunk_of(tok: int) -> int:
    """Chunk index containing global token `tok`."""
    for i, o in enumerate(_offs):
        if o <= tok < o + CHUNKS[i]:
            return i
    raise ValueError(tok)


def _build_nc() -> bass.Bass:
    nc = bass.Bass(dynamic_dma_scratch_size=2**17, num_swdge_queues=1)
    emb_d = nc.dram_tensor("emb8", [V, D], FP8, kind="ExternalInput")
    idx_d = nc.dram_tensor("idx", [128, T // 16], I16, kind="ExternalInput")
    cb_d = nc.dram_tensor("cls_b", [1, C], F32, kind="ExternalInput")
    yt_d = nc.dram_tensor("yt", [128, BPC * JT * CY], BF16, kind="ExternalInput")
    out_d = nc.dram_tensor("out", [1, BPC * C], F32, kind="ExternalOutput")

    mult = mybir.AluOpType.mult
    add = mybir.AluOpType.add
    EXP = mybir.ActivationFunctionType.Exp
    ACT = mybir.EngineType.Activation
    ESC = 1.0 / (EMB_SCALE * EMB_SCALE)
    IC0 = CHUNKS[0] // 16    # idx columns of chunk 0

    # --- index upload ---
    # Chunk 0's indices ride SP, the rest Activation. Emitted inside the
    # context so the tile scheduler sees them, then _hoist_idx_dmas moves
    # the two DMAs ahead of the entry barrier so their ~2.4us latency
    # chain overlaps the framework's entry sync.
    with tile.TileContext(nc) as tc:
        with (
            tc.tile_pool(name="const", bufs=1) as constp,
            tc.tile_pool(name="xp", bufs=1) as xp,
            tc.tile_pool(name="sp", bufs=2) as sp,
            tc.tile_pool(name="ps", bufs=1, space="PSUM") as pp,
        ):
            idx = constp.tile([128, T // 16], I16)
            nc.sync.dma_start(idx[:, 0:IC0], idx_d[:, 0:IC0])
            nc.engines[ACT].dma_start(idx[:, IC0:], idx_d[:, IC0:])
            yt = constp.tile([128, BPC, JT, CY], BF16)
            nc.sync.dma_start(yt[:, :, :, :], yt_d[:, :])
            cb = constp.tile([1, C], F32)
            nc.sync.dma_start(cb[:], cb_d[:, :])

            # --- transpose-gather the global token stream, chunk by chunk ---
            # Flat xt: chunk k (offset o, n tokens) occupies bytes
            # [4o, 4(o+n)) per partition, internally [v(2), i(n), e(2)]:
            # xt_k[p, v, i, e] = fp8 x_{o+i}[256 v + 2 p + e].
            xt = xp.tile([128, 4 * T], FP8, tag="xt")
            xv = []
            for k, (o, n) in enumerate(zip(_offs, CHUNKS)):
                xv.append(
                    xt[:, 4 * o:4 * (o + n)].rearrange(
                        "p (v i e) -> p v i e", v=2, e=2
                    )
                )
                gout = xt[:, 4 * o:4 * (o + n)].rearrange("p (a b) -> p a b", a=4)
                nc.gpsimd.dma_gather(
                    out_ap=gout,
                    in_ap=emb_d[:, :],
                    idxs_ap=idx[:, o // 16:(o + n) // 16],
                    num_idxs=n,
                    num_idxs_reg=n,
                    elem_size=D,
                    transpose=True,
                    queue_num=0,
                    single_packet=False,
                )

            spm_all = pp.tile([128, BPC, JT], F32, tag="spm")
            npm_all = pp.tile([1, BPC, CY], F32, tag="npm")
            ob = sp.tile([1, BPC * C], F32, tag="ob")

            def scores(b, j):
                """One 128-token tile's 4 accumulating dot-product matmuls:
                spm[:, b, j] (token 2048 b + 128 j + p) vs seq b's query."""
                t0 = _POS[(b, j)]
                k = _chunk_of(t0)
                i0 = t0 - _offs[k]
                kq = _chunk_of(_POS[(b, 0)])
                iq = _POS[(b, 0)] - _offs[kq]
                first = True
                for v in range(2):
                    for e in range(2):
                        nc.tensor.matmul(
                            spm_all[:, b, j:j + 1],
                            xv[k][:, v, i0:i0 + 128, e],
                            xv[kq][:, v, iq:iq + 1, e],
                            start=first,
                            stop=(v == 1 and e == 1),
                        )
                        first = False

            def exp_tiles(spm_slice, e_slice):
                nc.scalar.activation(e_slice, spm_slice, EXP, scale=ESC)

            def npm_b(b, e, j0, j1, start, stop):
                """Accumulating numerator matmuls for tiles [j0, j1); the
                ones column of yt folds Z into npm[:, b, C]."""
                for j in range(j0, j1):
                    nc.tensor.matmul(
                        npm_all[:, b, :], e[:, j - j0:j - j0 + 1], yt[:, b, j, :],
                        start=(start and j == j0), stop=(stop and j == j1 - 1),
                    )

            def finalize_b(b):
                """logits = npm[0:C] * (1/Z) + cls_b (Z is npm's ones col)."""
                rz = sp.tile([1, 1], F32, tag="rz")
                nc.vector.reciprocal(rz[:], npm_all[:, b, C:C + 1])
                nc.vector.scalar_tensor_tensor(
                    ob[:, b * C:(b + 1) * C], npm_all[:, b, 0:C],
                    rz[:], cb[:], op0=mult, op1=add,
                )

            # Score bursts chunk-major; softmax consumers slotted where
            # their inputs are already computed so PE/ACT/DVE never stall.
            #
            # Gate groups: maximal b-major runs of 128-token tiles whose
            # gather chunk is the same ("gate"). Each group gets one merged
            # exp activation over its contiguous psum columns as soon as
            # its gate chunk's scores exist; the numerator matmuls for a
            # group are emitted one burst later on the PE stream (by which
            # time the exp has long retired on the Activation engine).
            groups = []          # (gate, [(b, j0, j1), ...]) b-major runs
            for b in range(BPC):
                for j in range(JT):
                    g = _chunk_of(_POS[(b, j)])
                    if groups and groups[-1][0] == g:
                        runs = groups[-1][1]
                        if runs[-1][0] == b and runs[-1][2] == j:
                            runs[-1] = (b, runs[-1][1], j + 1)
                        else:
                            runs.append((b, j, j + 1))
                    else:
                        groups.append((g, [(b, j, j + 1)]))
            e_of = {}            # group index -> e tile

            def emit_exp(gi):
                gate, runs = groups[gi]
                ncols = sum(j1 - j0 for _, j0, j1 in runs)
                c0 = runs[0][0] * JT + runs[0][1]
                e_g = sp.tile([128, ncols], BF16, tag=f"eg{gi}", name=f"eg{gi}")
                e_of[gi] = e_g
                spm_flat = spm_all[:, :, :].rearrange("p b j -> p (b j)")
                exp_tiles(spm_flat[:, c0:c0 + ncols], e_g[:])

            def emit_npm(gi):
                gate, runs = groups[gi]
                col = 0
                for b, j0, j1 in runs:
                    first_of_b = (j0 == 0)
                    last_of_b = (j1 == JT)
                    npm_b(b, e_of[gi][:, col:col + (j1 - j0)], j0, j1,
                          first_of_b, last_of_b)
                    if last_of_b:
                        finalize_b(b)
                    col += j1 - j0

            def burst(k):
                for b, j in STREAM:
                    if _chunk_of(_POS[(b, j)]) == k:
                        scores(b, j)

            # Numerators lag their gate by TWO bursts: a one-burst lag puts
            # npm(gate k)'s wait on its exp ahead of burst k+2 in the
            # in-order PE stream, stalling the final score bursts ~1us.
            NK = len(CHUNKS)
            for k in range(NK):
                burst(k)
                # exp for groups gated by chunk k (ACT runs them as soon as
                # the burst's psum lands; ACT program order is gate order).
                for gi, (gate, _) in enumerate(groups):
                    if gate == k:
                        emit_exp(gi)
                for gi, (gate, _) in enumerate(groups):
                    if gate == k - 2:
                        emit_npm(gi)
            for gi, (gate, _) in enumerate(groups):
                if gate >= NK - 2:
                    emit_npm(gi)

            nc.sync.dma_start(out_d[:, :], ob[:])

    nc.finalize()
    _hoist_idx_dmas(nc)
    _bacc_postpasses(nc)
    _split_multiwaits(nc)
    return nc


def get_nc() -> bass.Bass:
    if "nc" not in _CACHE:
        _CACHE["nc"] = _build_nc()
    return _CACHE["nc"]


def _prep_tables(emb_table: np.ndarray, cls_w: np.ndarray):
    import ml_dtypes

    emb = np.asarray(emb_table, dtype=np.float32)
    y = (emb @ np.asarray(cls_w, dtype=np.float32).T).astype(ml_dtypes.bfloat16)
    emb8 = (emb * EMB_SCALE).astype(ml_dtypes.float8_e4m3fn)
    return emb8, y


def _build_yt(tokens: np.ndarray, y: np.ndarray) -> np.ndarray:
    """Per-core [128, BPC*JT*CY] bf16: yt[p, b, j, 0:C] = y[tok[b,128j+p], :]
    and yt[p, b, j, C] = 1 (token-major, matching the gather's score-column
    layout; the ones column folds Z into the numerator matmuls)."""
    import ml_dtypes

    out = np.empty((128, BPC, JT, CY), ml_dtypes.bfloat16)
    for b in range(BPC):
        out[:, b, :, 0:C] = y[tokens[b]].reshape(JT, 128, C).transpose(1, 0, 2)
    out[:, :, :, C] = 1.0
    return out.reshape(128, BPC * JT * CY)


def _build_idx(tokens: np.ndarray) -> np.ndarray:
    """Per-core [128, T//16] int16; STREAM-ordered token t sits at
    [16*g + t%16, t//16] for every 16-partition group g (the SWDGE
    TX core reads group 1; CoreSim reads group 0)."""
    toks16 = tokens.astype(np.int16)             # [BPC, S], values < 32000
    stream = np.concatenate(
        [toks16[b, 128 * j:128 * (j + 1)] for b, j in STREAM]
    )                                            # [T] in gather order
    slot = stream.reshape(T // 16, 16).T         # [16, T//16]
    return np.tile(slot, (8, 1))


def make_in_maps(tokens, emb_table, cls_w, cls_b):
    tokens = np.asarray(tokens)
    emb8, y = _prep_tables(emb_table, cls_w)
    cb = np.ascontiguousarray(np.asarray(cls_b, dtype=np.float32)).reshape(1, C)
    in_maps = []
    for core in range(N_CORES):
        ct = tokens[core * BPC:(core + 1) * BPC]
        in_maps.append(
            {
                "emb8": emb8,
                "idx": _build_idx(ct),
                "yt": _build_yt(ct, y),
                "cls_b": cb,
            }
        )
    return in_maps


def kernel(tokens, emb_table, cls_w, cls_b) -> np.ndarray:
    nc = get_nc()
    in_maps = make_in_maps(tokens, emb_table, cls_w, cls_b)
    res = run_bass_kernel_spmd(nc, in_maps, core_ids=list(range(N_CORES)))
    outs = [res.results[c]["out"].reshape(BPC, C) for c in range(N_CORES)]
    return np.concatenate(outs, axis=0).astype(np.float32)


# revision 47
# speedup vs baseline: 1.4084x; 1.0009x over previous
"""Trainium2 Bass kernel for nn_AttentionModel (sparse_attention).

Reference computation:
    x = emb_table[tokens]                  # [B,S,D]
    scores = x @ x^T per batch             # [B,S,S]
    out = softmax(scores) @ x              # [B,S,D]
    logits = out[:, 0, :] @ cls_w.T + cls_b

Only row 0 of the attention output is used, and that row only ever meets
cls_w, so per batch element the whole model reduces to

    q = x[0]
    s_t = <x_t, q>                 (2048 dot products of length 512)
    e = exp(s);  Z = sum(e)
    logits_c = sum_t e_t * y[tok_t, c] / Z + b_c,   y = emb_table @ cls_w^T

Device strategy (data-parallel over batch, 8 cores x 4 sequences = 8192
tokens per core, treated as one global token stream):

  * The table is uploaded as fp8(emb*32) [32000, 512] (512B rows).
    dma_gather(transpose=True) fetches rows in d-major layout; within a
    chunk of n tokens the SBUF bytes are [p, v(2), i(n), e(2)] =
    fp8 x_i[256*v + 2*p + e]. Token 2048*b doubles as sequence b's query.
  * Gathers use single_packet=False on ONE SWDGE queue: that combination
    is correct on hardware for arbitrarily large num_idxs (verified to
    8192), unlike single_packet=True (crashes above 512 indices) or
    multi-queue single_packet=False (queues corrupt each other). Large
    chunks amortize the ~1us SWDGE fixed cost per instruction, so the
    gather DMA stream (4 MiB/core at the hard 360 GB/s ceiling) is the
    pacer, not Q7 descriptor generation.
  * Chunk sizes ramp [1280, 2432, 2432, 1408, 640]: the first chunk is
    the smallest that keeps descriptor generation ahead of the DMA
    engines (994 + 0.34 n_next <= 1.42 n ns), chunk boundaries align
    with sequence ends so each sequence's softmax is gated as early as
    possible, and the small last chunk keeps the end-of-stream softmax
    work to five tiles of one sequence.
  * The index upload (split SP / Activation HWDGE queues) is hoisted
    ahead of the TileContext entry barrier by a post-pass, so its
    ~2.4us latency chain overlaps the framework's entry sync instead of
    following it.
  * Scores run on the PE as stationary-weight matmuls (contraction dim d
    on partitions, 128-token output columns), psum-accumulated over
    (v, e). Emission is chunk-major with softmax consumers slotted
    between score bursts at points where their exp results are already
    available, so the in-order PE stream never stalls.
  * exp happens on the scalar engine reading psum (scale folds away the
    fp8 *32 scaling); sequences gated by the same chunk share one
    activation instruction. Z rides the numerator matmuls as a third
    ones-column in the host-prepped y table, so sum_t e_t * (y0, y1, 1)
    is one accumulating [128,1]x[128,3] matmul chain per sequence and
    logits finish with a DVE reciprocal + multiply-add. The last
    sequence's softmax splits at the final chunk boundary so little work
    trails the last gather, and numerator chains lag their score bursts
    by two chunks so the in-order PE stream never stalls a burst.
  * All four sequences' logits collect in one [1, 8] tile and leave in a
    single output DMA.
"""

import numpy as np

import bass_rust

import concourse.bass as bass
import concourse.mybir as mybir
import concourse.tile as tile
from concourse.bass_utils import run_bass_kernel_spmd


def _split_multiwaits(nc: bass.Bass) -> None:
    """Workaround for the walrus build in this container, which rejects
    instructions carrying more than one sync-wait command ("Too many sync
    wait commands" / "ISA wrong length" in CoreV3GenImpl setupSyncWait).

    Moves each instruction's sync waits onto dedicated single-wait NOPs
    inserted right before it on the same engine stream (bass_nofuse so
    walrus's nop-fusion can't merge them back)."""
    counter = 0
    fn = nc.m.functions[0]
    for bb in fn.blocks:
        insts = bb.instructions
        new_list = []
        changed = False
        for inst in insts:
            si = inst.sync_info
            waits = list(si.on_wait) if si is not None else []
            if waits:
                for w in waits:
                    counter += 1
                    new_list.append(
                        mybir.InstNoOp(
                            name=f"waitnop-{counter}",
                            engine=inst.engine,
                            ins=[],
                            outs=[],
                            bass_nofuse=True,
                            sync_info=bass_rust.SyncInfo(on_wait=[w], on_update=[]),
                        )
                    )
                inst.sync_info = bass_rust.SyncInfo(
                    on_wait=[], on_update=list(si.on_update)
                )
                changed = True
            new_list.append(inst)
        if changed:
            bb.instructions = new_list


def _hoist_idx_dmas(nc: bass.Bass) -> None:
    """Move the two index-upload DMACopies (they read the DRAM tensor
    named "idx") ahead of the TileContext entry barrier: out of their
    scheduled block and into the preamble block, right after their
    engine's RegisterMove run. Their framework waits are stripped (they
    are the first writes to the idx tile); their completion-sem
    increments stay, which is all the gathers wait on."""
    fn = nc.m.functions[0]
    blocks = fn.blocks
    moved = []
    for bb in blocks:
        keep = []
        for inst in bb.instructions:
            is_idx_dma = isinstance(inst, mybir.InstDMACopy) and any(
                getattr(ap, "memref", None) == "idx" for ap in list(inst.ins)
            )
            if is_idx_dma:
                si = inst.sync_info
                ups = list(si.on_update) if si is not None else []
                inst.sync_info = bass_rust.SyncInfo(on_wait=[], on_update=ups)
                moved.append(inst)
            else:
                keep.append(inst)
        if len(keep) != len(bb.instructions):
            bb.instructions = keep
    assert len(moved) == 2, f"expected 2 idx DMAs, found {len(moved)}"
    b0 = blocks[0]
    insts = list(b0.instructions)
    for inst in moved:
        last_rm = max(
            i for i, x in enumerate(insts)
            if isinstance(x, mybir.InstRegisterMove) and x.engine == inst.engine
        )
        insts.insert(last_rm + 1, inst)
    b0.instructions = insts


def _bacc_postpasses(nc: bass.Bass) -> None:
    """GPSIMD extended instructions (InstDMAGatherAnt) need their Q7 library
    load inserted and ISA payload bytes generated — Bacc does this in
    compile(); plain Bass does not."""
    from concourse.library_config import all_libraries, standard

    mask: dict = {}
    for lib in all_libraries:
        for it in lib.instructions:
            mask[it] = mask.get(it, 0) | (1 << lib.index)
    bass_rust.insert_library_loads(nc, mask, len(all_libraries), standard.index)
    mybir.codegen_inst_isa_subclasses(nc)


B, S, D, V, C = 32, 2048, 512, 32000, 2
CY = C + 1                  # y columns + the folded-Z ones column
N_CORES = 8
BPC = B // N_CORES          # sequences per core
T = BPC * S                 # tokens per core (global stream)
JT = S // 128               # 16 token tiles per sequence
EMB_SCALE = 32.0            # emb is quantized as fp8(emb*32); scores carry 32^2

# STREAM lists 128-token tiles (b, j) in gather order (natural order
# measured fastest end-to-end); CHUNKS slices the stream into gather
# instructions (each a multiple of 128, summing to T; sized so SWDGE
# descriptor generation stays ahead of the DMA engines:
# 994 + 0.34 n_next <= 1.42 n ns, with boundaries on sequence ends).
STREAM = [(b, j) for b in range(BPC) for j in range(JT)]
_POS = {bj: 128 * i for i, bj in enumerate(STREAM)}   # (b, j) -> stream token
CHUNKS = [1280, 2432, 2432, 1408, 640]
assert sum(CHUNKS) == T and all(c % 128 == 0 for c in CHUNKS)
_offs = [sum(CHUNKS[:i]) for i in range(len(CHUNKS))]

F32 = mybir.dt.float32
BF16 = mybir.dt.bfloat16
FP8 = mybir.dt.float8e4
I16 = mybir.dt.int16

_CACHE: dict = {}


def _chunk_of(tok: int) -> int:
    """Chunk index containing global token `tok`."""
    for i, o in enumerate(_offs):
        if o <= tok < o + CHUNKS[i]:
            return i
    raise ValueError(tok)


def _build_nc() -> bass.Bass:
    nc = bass.Bass(dynamic_dma_scratch_size=2**17, num_swdge_queues=1)
    emb_d = nc.dram_tensor("emb8", [V, D], FP8, kind="ExternalInput")
    idx_d = nc.dram_tensor("idx", [128, T // 16], I16, kind="ExternalInput")
    cb_d = nc.dram_tensor("cls_b", [1, C], F32, kind="ExternalInput")
    yt_d = nc.dram_tensor("yt", [128, BPC * JT * CY], BF16, kind="ExternalInput")
    out_d = nc.dram_tensor("out", [1, BPC * C], F32, kind="ExternalOutput")

    mult = mybir.AluOpType.mult
    add = mybir.AluOpType.add
    EXP = mybir.ActivationFunctionType.Exp
    ACT = mybir.EngineType.Activation
    ESC = 1.0 / (EMB_SCALE * EMB_SCALE)
    IC0 = CHUNKS[0] // 16    # idx columns of chunk 0

    # --- index upload ---
    # Chunk 0's indices ride SP, the rest Activation. Emitted inside the
    # context so the tile scheduler sees them, then _hoist_idx_dmas moves
    # the two DMAs ahead of the entry barrier so their ~2.4us latency
    # chain overlaps the framework's entry sync.
    with tile.TileContext(nc) as tc:
        with (
            tc.tile_pool(name="const", bufs=1) as constp,
            tc.tile_pool(name="xp", bufs=1) as xp,
            tc.tile_pool(name="sp", bufs=2) as sp,
            tc.tile_pool(name="ps", bufs=1, space="PSUM") as pp,
        ):
            idx = constp.tile([128, T // 16], I16)
            nc.sync.dma_start(idx[:, 0:IC0], idx_d[:, 0:IC0])
            nc.engines[ACT].dma_start(idx[:, IC0:], idx_d[:, IC0:])
            yt = constp.tile([128, BPC, JT, CY], BF16)
            nc.sync.dma_start(yt[:, :, :, :], yt_d[:, :])
            cb = constp.tile([1, C], F32)
            nc.sync.dma_start(cb[:], cb_d[:, :])

            # --- transpose-gather the global token stream, chunk by chunk ---
            # Flat xt: chunk k (offset o, n tokens) occupies bytes
            # [4o, 4(o+n)) per partition, internally [v(2), i(n), e(2)]:
            # xt_k[p, v, i, e] = fp8 x_{o+i}[256 v + 2 p + e].
            xt = xp.tile([128, 4 * T], FP8, tag="xt")
            xv = []
            for k, (o, n) in enumerate(zip(_offs, CHUNKS)):
                xv.append(
                    xt[:, 4 * o:4 * (o + n)].rearrange(
                        "p (v i e) -> p v i e", v=2, e=2
                    )
                )
                gout = xt[:, 4 * o:4 * (o + n)].rearrange("p (a b) -> p a b", a=4)
                nc.gpsimd.dma_gather(
                    out_ap=gout,
                    in_ap=emb_d[:, :],
                    idxs_ap=idx[:, o // 16:(o + n) // 16],
                    num_idxs=n,
                    num_idxs_reg=n,
                    elem_size=D,
                    transpose=True,
                    queue_num=0,
                    single_packet=False,
                )

            spm_all = pp.tile([128, BPC, JT], F32, tag="spm")
            npm_all = pp.tile([1, BPC, CY], F32, tag="npm")
            ob = sp.tile([1, BPC * C], F32, tag="ob")

            def scores(b, j):
                """One 128-token tile's 4 accumulating dot-product matmuls:
                spm[:, b, j] (token 2048 b + 128 j + p) vs seq b's query."""
                t0 = _POS[(b, j)]
                k = _chunk_of(t0)
                i0 = t0 - _offs[k]
                kq = _chunk_of(_POS[(b, 0)])
                iq = _POS[(b, 0)] - _offs[kq]
                first = True
                for v in range(2):
                    for e in range(2):
                        nc.tensor.matmul(
                            spm_all[:, b, j:j + 1],
                            xv[k][:, v, i0:i0 + 128, e],
                            xv[kq][:, v, iq:iq + 1, e],
                            start=first,
                            stop=(v == 1 and e == 1),
                        )
                        first = False

            def exp_tiles(spm_slice, e_slice):
                nc.scalar.activation(e_slice, spm_slice, EXP, scale=ESC)

            def npm_b(b, e, j0, j1, start, stop):
                """Accumulating numerator matmuls for tiles [j0, j1); the
                ones column of yt folds Z into npm[:, b, C]."""
                for j in range(j0, j1):
                    nc.tensor.matmul(
                        npm_all[:, b, :], e[:, j - j0:j - j0 + 1], yt[:, b, j, :],
                        start=(start and j == j0), stop=(stop and j == j1 - 1),
                    )

            def finalize_b(b):
                """logits = npm[0:C] * (1/Z) + cls_b (Z is npm's ones col)."""
                rz = sp.tile([1, 1], F32, tag="rz")
                nc.vector.reciprocal(rz[:], npm_all[:, b, C:C + 1])
                nc.vector.scalar_tensor_tensor(
                    ob[:, b * C:(b + 1) * C], npm_all[:, b, 0:C],
                    rz[:], cb[:], op0=mult, op1=add,
                )

            # Score bursts chunk-major; softmax consumers slotted where
            # their inputs are already computed so PE/ACT/DVE never stall.
            #
            # Gate groups: maximal b-major runs of 128-token tiles whose
            # gather chunk is the same ("gate"). Each group gets one merged
            # exp activation over its contiguous psum columns as soon as
            # its gate chunk's scores exist; the numerator matmuls for a
            # group are emitted one burst later on the PE stream (by which
            # time the exp has long retired on the Activation engine).
            groups = []          # (gate, [(b, j0, j1), ...]) b-major runs
            for b in range(BPC):
                for j in range(JT):
                    g = _chunk_of(_POS[(b, j)])
                    if groups and groups[-1][0] == g:
                        runs = groups[-1][1]
                        if runs[-1][0] == b and runs[-1][2] == j:
                            runs[-1] = (b, runs[-1][1], j + 1)
                        else:
                            runs.append((b, j, j + 1))
                    else:
                        groups.append((g, [(b, j, j + 1)]))
            e_of = {}            # group index -> e tile

            def emit_exp(gi):
                gate, runs = groups[gi]
                ncols = sum(j1 - j0 for _, j0, j1 in runs)
                c0 = runs[0][0] * JT + runs[0][1]
                e_g = sp.tile([128, ncols], BF16, tag=f"eg{gi}", name=f"eg{gi}")
                e_of[gi] = e_g
                spm_flat = spm_all[:, :, :].rearrange("p b j -> p (b j)")
                exp_tiles(spm_flat[:, c0:c0 + ncols], e_g[:])

            def emit_npm(gi):
                gate, runs = groups[gi]
                col = 0
                for b, j0, j1 in runs:
                    first_of_b = (j0 == 0)
                    last_of_b = (j1 == JT)
                    npm_b(b, e_of[gi][:, col:col + (j1 - j0)], j0, j1,
                          first_of_b, last_of_b)
                    if last_of_b:
                        finalize_b(b)
                    col += j1 - j0

            def burst(k):
                for b, j in STREAM:
                    if _chunk_of(_POS[(b, j)]) == k:
                        scores(b, j)

            # Numerators lag their gate by TWO bursts: a one-burst lag puts
            # npm(gate k)'s wait on its exp ahead of burst k+2 in the
            # in-order PE stream, stalling the final score bursts ~1us.
            NK = len(CHUNKS)
            for k in range(NK):
                burst(k)
                # exp for groups gated by chunk k (ACT runs them as soon as
                # the burst's psum lands; ACT program order is gate order).
                for gi, (gate, _) in enumerate(groups):
                    if gate == k:
                        emit_exp(gi)
                for gi, (gate, _) in enumerate(groups):
                    if gate == k - 2:
                        emit_npm(gi)
            for gi, (gate, _) in enumerate(groups):
                if gate >= NK - 2:
                    emit_npm(gi)

            nc.sync.dma_start(out_d[:, :], ob[:])

    nc.finalize()
    _hoist_idx_dmas(nc)
    _bacc_postpasses(nc)
    _split_multiwaits(nc)
    return nc


def get_nc() -> bass.Bass:
    if "nc" not in _CACHE:
        _CACHE["nc"] = _build_nc()
    return _CACHE["nc"]


def _prep_tables(emb_table: np.ndarray, cls_w: np.ndarray):
    import ml_dtypes

    emb = np.asarray(emb_table, dtype=np.float32)
    y = (emb @ np.asarray(cls_w, dtype=np.float32).T).astype(ml_dtypes.bfloat16)
    emb8 = (emb * EMB_SCALE).astype(ml_dtypes.float8_e4m3fn)
    return emb8, y


def _build_yt(tokens: np.ndarray, y: np.ndarray) -> np.ndarray:
    """Per-core [128, BPC*JT*CY] bf16: yt[p, b, j, 0:C] = y[tok[b,128j+p], :]
    and yt[p, b, j, C] = 1 (token-major, matching the gather's score-column
    layout; the ones column folds Z into the numerator matmuls)."""
    import ml_dtypes

    out = np.empty((128, BPC, JT, CY), ml_dtypes.bfloat16)
    for b in range(BPC):
        out[:, b, :, 0:C] = y[tokens[b]].reshape(JT, 128, C).transpose(1, 0, 2)
    out[:, :, :, C] = 1.0
    return out.reshape(128, BPC * JT * CY)


def _build_idx(tokens: np.ndarray) -> np.ndarray:
    """Per-core [128, T//16] int16; STREAM-ordered token t sits at
    [16*g + t%16, t//16] for every 16-partition group g (the SWDGE
    TX core reads group 1; CoreSim reads group 0)."""
    toks16 = tokens.astype(np.int16)             # [BPC, S], values < 32000
    stream = np.concatenate(
        [toks16[b, 128 * j:128 * (j + 1)] for b, j in STREAM]
    )                                            # [T] in gather order
    slot = stream.reshape(T // 16, 16).T         # [16, T//16]
    return np.tile(slot, (8, 1))


def make_in_maps(tokens, emb_table, cls_w, cls_b):
    tokens = np.asarray(tokens)
    emb8, y = _prep_tables(emb_table, cls_w)
    cb = np.ascontiguousarray(np.asarray(cls_b, dtype=np.float32)).reshape(1, C)
    in_maps = []
    for core in range(N_CORES):
        ct = tokens[core * BPC:(core + 1) * BPC]
        in_maps.append(
            {
                "emb8": emb8,
                "idx": _build_idx(ct),
                "yt": _build_yt(ct, y),
                "cls_b": cb,
            }
        )
    return in_maps


def kernel(tokens, emb_table, cls_w, cls_b) -> np.ndarray:
    nc = get_nc()
    in_maps = make_in_maps(tokens, emb_table, cls_w, cls_b)
    res = run_bass_kernel_spmd(nc, in_maps, core_ids=list(range(N_CORES)))
    outs = [res.results[c]["out"].reshape(BPC, C) for c in range(N_CORES)]
    return np.concatenate(outs, axis=0).astype(np.float32)
